# revision 19
# baseline (speedup 1.0000x reference)
"""Trainium2 Bass kernel for nn_Clustering_20435454394868.

Reference structure (see problem): three sequence-axis shrink reductions
  Ks/Vs/Qs[b,h,m,d] = sum_k W[m,k] * X[b,h,k,d] + bias[m]      (m=8, k=4096)
feed a tiny cluster/softmax head producing a scalar loss, and the final
context collapses to a per-(b,h) broadcast of mean_m(Vs) over all 4096
query rows (the reference's duplicate-index scatter is won by a fully
tril-masked row, so every softmax row is uniform 1/8).

Device work (memory-bound part): stream Q,K,V (192MB total, 24MB/core,
data-parallel over the 64 (b,h) pairs across 8 cores) through TensorE
matmul reductions. Everything O(KB) afterwards runs on host in float64.
"""

import sys

if "/opt/trn_rl_repo" not in sys.path:
    sys.path.insert(0, "/opt/trn_rl_repo")

import numpy as np

import concourse.bacc as bacc
import concourse.bass as bass
import concourse.mybir as mybir
import concourse.tile as tile
from concourse import bass_utils

N_CORES = 8
B, H, L, DK = 4, 16, 4096, 64
KTOT = 4096                  # contraction length (seq axis)
M = 8                        # shrink output size (LOG_LK = LOG_L = 8)
NPAIR = B * H                # 64 (b,h) pairs
PPC = NPAIR // N_CORES       # 8 pairs per core
G = 4                        # pairs batched per matmul (rhs free = G*DK = 256)
NG = PPC // G                # groups per tensor per core
P = 128                      # SBUF partitions (contraction tile)
J = KTOT // P                # 32 sub-steps per contraction
F32 = mybir.dt.float32

# Matmul mode: "mixed" (default) runs the V reduction in two-pass fp32
# (full fp32 accuracy for the context output) and Q/K in one-pass fp32r
# (4x less PE time; they only feed the scalar loss, where 1e-4-level
# rounding is irrelevant). "f32r_raw" is all-fp32r, "f32" all-fp32.
MM_MODE = "mixed"

_NC_CACHE = {}

# (group pairs per matmul, fuse group load into one DMA, alternate the two
# HWDGE rings, iterations of the whole body -- >1 only for timing runs)
# HW-tuned: 2-pair groups (2MB loads, N=128 matmuls) hide the PE tail best;
# alternating sync/scalar HWDGE rings pipelines DMA fixed costs; weights go
# via gpsimd (SWDGE) so they never delay the first input DMA.
GROUP = 2
FUSED_DMA = True
ALT_RINGS = True
N_ITERS = 1


def _build_nc(mode=None, group=None, fused=None, alt=None, n_iters=None,
              loop_reps=1):
    mode = mode or MM_MODE
    group = GROUP if group is None else group
    fused = FUSED_DMA if fused is None else fused
    alt = ALT_RINGS if alt is None else alt
    n_iters = N_ITERS if n_iters is None else n_iters
    key = (mode, group, fused, alt, n_iters, loop_reps)
    if key in _NC_CACHE:
        return _NC_CACHE[key]
    ng = PPC // group

    F32R = mybir.dt.float32r
    # Per-tensor dtype and processing order. V first: its fp32 matmuls are
    # slower than its own DMA and need the Q/K groups' PE slack to hide.
    # Indices into the O output: 0=V, 1=Q, 2=K (host unscrambles).
    if mode == "mixed":
        tensor_dts = [F32, F32R, F32R]
    elif mode == "f32":
        tensor_dts = [F32, F32, F32]
    else:
        tensor_dts = [F32R, F32R, F32R]

    nc = bacc.Bacc("TRN2", target_bir_lowering=False, debug=False,
                   num_devices=N_CORES)
    Xv = nc.dram_tensor("Xv", [PPC, KTOT, DK], tensor_dts[0], kind="ExternalInput")
    Xq = nc.dram_tensor("Xq", [PPC, KTOT, DK], tensor_dts[1], kind="ExternalInput")
    Xk = nc.dram_tensor("Xk", [PPC, KTOT, DK], tensor_dts[2], kind="ExternalInput")
    W = nc.dram_tensor("W", [3, KTOT, M], F32, kind="ExternalInput")
    O = nc.dram_tensor("O", [3, ng, M, group, DK], F32, kind="ExternalOutput")

    if alt == "3ring":
        dma_engines = [nc.sync, nc.scalar, nc.gpsimd]
    elif alt:
        dma_engines = [nc.sync, nc.scalar]  # the two HWDGE rings
    else:
        dma_engines = [nc.sync]

    with tile.TileContext(nc) as tc:
        with (
            tc.tile_pool(name="wp", bufs=1) as wp,
            tc.tile_pool(name="xp", bufs=4) as xp,
            tc.tile_pool(name="op", bufs=2) as op,
            tc.tile_pool(name="pp", bufs=2, space=bass.MemorySpace.PSUM) as pp,
        ):
            # [128, 3*J*M]: wt[p, (t, j*M+m)] = W[t, p*J+j, m], one SWDGE
            # DMA (off the two HWDGE input rings). W order matches (V, Q, K).
            wts = []
            for t in range(3):
                wt = wp.tile([P, J * M], tensor_dts[t], tag=f"w{t}")
                nc.gpsimd.dma_start(
                    wt[:], W[t].rearrange("(p j) m -> p (j m)", p=P))
                wts.append(wt)

            import contextlib

            state = {"dma_i": 0}

            def body():
                # Sequential per tensor, V first. Interleaving V's fp32
                # matmul chains with f32r chains (round-robin order) breaks
                # the fp32 two-pass pairing on the PE and silently degrades
                # V's precision to ~4e-5 rms -- keep each tensor's chains
                # contiguous. V's slower fp32 PE bursts overlap the Q/K
                # loads that run ahead on the other DMA ring/buffers.
                for t, Xd in enumerate((Xv, Xq, Xk)):
                    for g in range(ng):
                        # Per pair a [128, J*DK] block: partition p holds seq
                        # rows p*J..p*J+J-1, so every DMA moves 8KB-contiguous
                        # runs per partition (max descriptor efficiency).
                        xg = xp.tile([P, group * J * DK], tensor_dts[t],
                                     tag=f"x{tensor_dts[t]}")
                        if fused:
                            eng = dma_engines[state["dma_i"] % len(dma_engines)]
                            state["dma_i"] += 1
                            eng.dma_start(
                                xg[:].rearrange("p (q f) -> p q f", q=group),
                                Xd[g * group:(g + 1) * group].rearrange(
                                    "q (p j) d -> p q (j d)", p=P),
                            )
                        else:
                            for q in range(group):
                                eng = dma_engines[state["dma_i"] % len(dma_engines)]
                                state["dma_i"] += 1
                                eng.dma_start(
                                    xg[:, q * J * DK:(q + 1) * J * DK],
                                    Xd[g * group + q].rearrange(
                                        "(p j) d -> p (j d)", p=P),
                                )
                        xg4 = xg.rearrange("p (q j d) -> p q j d", q=group, j=J)
                        ps = pp.tile([M, group * DK], F32)
                        for j in range(J):
                            # out[m,(q,d)] += sum_p W[t,p*J+j,m] * X[q,p*J+j,d]
                            nc.tensor.matmul(
                                ps[:],
                                wts[t][:, j * M:(j + 1) * M],
                                xg4[:, :, j, :],
                                start=(j == 0),
                                stop=(j == J - 1),
                            )
                        so = op.tile([M, group * DK], F32)
                        nc.vector.tensor_copy(so[:], ps[:])
                        nc.sync.dma_start(O[t, g], so[:])

            loop_cm = (tc.For_i(0, loop_reps, 1) if loop_reps > 1
                       else contextlib.nullcontext())
            with loop_cm:
                for it in range(n_iters):
                    body()

    nc.compile()
    _NC_CACHE[key] = nc
    return nc


# test.py introspection: exec_time_ns of the last run when tracing is on.
LAST_RESULTS = None


def _softmax(x, axis=-1):
    x = x - x.max(axis=axis, keepdims=True)
    e = np.exp(x)
    return e / e.sum(axis=axis, keepdims=True)


def _log_softmax(x, axis=-1):
    x = x - x.max(axis=axis, keepdims=True)
    return x - np.log(np.exp(x).sum(axis=axis, keepdims=True))


def kernel(**inputs):
    global LAST_RESULTS
    nc = _build_nc()

    f32 = np.float32
    Q = np.ascontiguousarray(inputs["Q"], dtype=f32).reshape(NPAIR, KTOT, DK)
    K = np.ascontiguousarray(inputs["K"], dtype=f32).reshape(NPAIR, KTOT, DK)
    V = np.ascontiguousarray(inputs["V"], dtype=f32).reshape(NPAIR, KTOT, DK)

    # device-side tensor order is (V, Q, K) -- see _build_nc
    Wt = np.empty((3, KTOT, M), dtype=f32)
    Wt[0] = np.ascontiguousarray(inputs["shrink_v_w"], dtype=f32).T
    Wt[1] = np.ascontiguousarray(inputs["shrink_q_w"], dtype=f32).T
    Wt[2] = np.ascontiguousarray(inputs["shrink_k_w"], dtype=f32).T

    in_maps = [
        {
            "Xv": V[c * PPC:(c + 1) * PPC],
            "Xq": Q[c * PPC:(c + 1) * PPC],
            "Xk": K[c * PPC:(c + 1) * PPC],
            "W": Wt,
        }
        for c in range(N_CORES)
    ]

    res = bass_utils.run_bass_kernel_spmd(nc, in_maps, list(range(N_CORES)))
    LAST_RESULTS = res

    # [cores, 3, ng, M, group, DK] -> [3, B, H, M, DK]; t order is (V, Q, K)
    Os = np.stack([res.results[c]["O"] for c in range(N_CORES)])
    per_t = Os.transpose(1, 0, 2, 4, 3, 5).reshape(3, NPAIR, M, DK)
    per_t = per_t.reshape(3, B, H, M, DK).astype(np.float64)

    Vs = per_t[0] + np.asarray(inputs["shrink_v_b"], np.float64)[None, None, :, None]
    Qs = per_t[1] + np.asarray(inputs["shrink_q_b"], np.float64)[None, None, :, None]
    Ks = per_t[2] + np.asarray(inputs["shrink_k_b"], np.float64)[None, None, :, None]

    pk_w = np.asarray(inputs["pk_w"], np.float64)
    pk_b = np.asarray(inputs["pk_b"], np.float64)
    pq_w = np.asarray(inputs["pq_w"], np.float64)
    pq_b = np.asarray(inputs["pq_b"], np.float64)
    qp_w = np.asarray(inputs["qp_w"], np.float64)
    qp_b = np.asarray(inputs["qp_b"], np.float64)
    kp_w = np.asarray(inputs["kp_w"], np.float64)
    kp_b = np.asarray(inputs["kp_b"], np.float64)

    ckp = np.maximum(Ks.reshape(B, -1) @ pk_w.T + pk_b, 0.0)
    cqp = np.maximum(Qs.reshape(B, -1) @ pq_w.T + pq_b, 0.0)
    cluster_q = _softmax(cqp @ qp_w.T + qp_b, axis=-1)
    cluster_k = _softmax(ckp @ kp_w.T + kp_b, axis=-1)

    mu = cluster_q.mean(axis=0)
    sigma = np.logaddexp(0.0, cluster_k.std(axis=0, ddof=1))  # softplus
    logp = (-0.5 * np.square((cluster_k - mu) / sigma)
            - np.log(sigma) - 0.5 * np.log(2.0 * np.pi))
    ce = np.mean(-np.sum(cluster_q * _log_softmax(cluster_q, axis=-1), axis=-1))
    loss = np.asarray(-np.mean(logp) + ce, dtype=f32)

    # Every attention row is uniform 1/8 (see module docstring), so the
    # context is mean_m(Vs) broadcast over all L query positions.
    ctx_row = Vs.mean(axis=2).astype(f32)  # [B, H, DK]
    context = np.ascontiguousarray(
        np.broadcast_to(ctx_row[:, :, None, :], (B, H, L, DK)))

    return context, loss


# revision 21
# speedup vs baseline: 1.2802x; 1.2802x over previous
"""Trainium2 Bass kernel for nn_Clustering_20435454394868.

Reference structure (see problem): three sequence-axis shrink reductions
  Ks/Vs/Qs[b,h,m,d] = sum_k W[m,k] * X[b,h,k,d] + bias[m]      (m=8, k=4096)
feed a tiny cluster/softmax head producing a scalar loss, and the final
context collapses to a per-(b,h) broadcast of mean_m(Vs) over all 4096
query rows (the reference's duplicate-index scatter is won by a fully
tril-masked row, so every softmax row is uniform 1/8).

Device work (memory-bound part): stream Q,K,V (192MB total, 24MB/core,
data-parallel over the 64 (b,h) pairs across 8 cores) through TensorE
matmul reductions. Everything O(KB) afterwards runs on host in float64.
"""

import sys

if "/opt/trn_rl_repo" not in sys.path:
    sys.path.insert(0, "/opt/trn_rl_repo")

import numpy as np

import concourse.bacc as bacc
import concourse.bass as bass
import concourse.mybir as mybir
import concourse.tile as tile
from concourse import bass_utils

N_CORES = 8
B, H, L, DK = 4, 16, 4096, 64
KTOT = 4096                  # contraction length (seq axis)
M = 8                        # shrink output size (LOG_LK = LOG_L = 8)
NPAIR = B * H                # 64 (b,h) pairs
PPC = NPAIR // N_CORES       # 8 pairs per core
P = 128                      # SBUF partitions (contraction tile)
J = KTOT // P                # 32 sub-steps per contraction
F32 = mybir.dt.float32

# Matmul mode: "mixed" (default) runs the V reduction in two-pass fp32
# (full fp32 accuracy for the context output) and Q/K in one-pass fp32r
# (4x less PE time; they only feed the scalar loss, where 1e-4-level
# rounding is irrelevant). "f32r_raw" is all-fp32r, "f32" all-fp32.
MM_MODE = "mixed"

_NC_CACHE = {}

# (group pairs per matmul, fuse group load into one DMA, alternate the two
# HWDGE rings, iterations of the whole body -- >1 only for timing runs)
# HW-tuned: 2-pair groups (2MB loads, N=128 matmuls) hide the PE tail best;
# alternating sync/scalar HWDGE rings pipelines DMA fixed costs; weights go
# via gpsimd (SWDGE) so they never delay the first input DMA.
GROUP = 2
FUSED_DMA = True
ALT_RINGS = True
N_ITERS = 1


def _build_nc(mode=None, group=None, fused=None, alt=None, n_iters=None,
              loop_reps=1):
    mode = mode or MM_MODE
    group = GROUP if group is None else group
    fused = FUSED_DMA if fused is None else fused
    alt = ALT_RINGS if alt is None else alt
    n_iters = N_ITERS if n_iters is None else n_iters
    key = (mode, group, fused, alt, n_iters, loop_reps)
    if key in _NC_CACHE:
        return _NC_CACHE[key]
    ng = PPC // group

    F32R = mybir.dt.float32r
    # Per-tensor dtype and processing order. V first: its fp32 matmuls are
    # slower than its own DMA and need the Q/K groups' PE slack to hide.
    # Indices into the O output: 0=V, 1=Q, 2=K (host unscrambles).
    if mode == "mixed":
        tensor_dts = [F32, F32R, F32R]
    elif mode == "f32":
        tensor_dts = [F32, F32, F32]
    else:
        tensor_dts = [F32R, F32R, F32R]

    nc = bacc.Bacc("TRN2", target_bir_lowering=False, debug=False,
                   num_devices=N_CORES)
    Xv = nc.dram_tensor("Xv", [PPC, KTOT, DK], tensor_dts[0], kind="ExternalInput")
    Xq = nc.dram_tensor("Xq", [PPC, KTOT, DK], tensor_dts[1], kind="ExternalInput")
    Xk = nc.dram_tensor("Xk", [PPC, KTOT, DK], tensor_dts[2], kind="ExternalInput")
    W = nc.dram_tensor("W", [3, KTOT, M], F32, kind="ExternalInput")
    O = nc.dram_tensor("O", [3, ng, M, group, DK], F32, kind="ExternalOutput")

    if alt == "3ring":
        dma_engines = [nc.sync, nc.scalar, nc.gpsimd]
    elif alt:
        dma_engines = [nc.sync, nc.scalar]  # the two HWDGE rings
    else:
        dma_engines = [nc.sync]

    with tile.TileContext(nc) as tc:
        with (
            tc.tile_pool(name="wp", bufs=1) as wp,
            tc.tile_pool(name="xp", bufs=4) as xp,
            tc.tile_pool(name="op", bufs=2) as op,
            tc.tile_pool(name="pp", bufs=2, space=bass.MemorySpace.PSUM) as pp,
        ):
            # [128, 3*J*M]: wt[p, (t, j*M+m)] = W[t, p*J+j, m], one SWDGE
            # DMA (off the two HWDGE input rings). W order matches (V, Q, K).
            wts = []
            for t in range(3):
                wt = wp.tile([P, J * M], tensor_dts[t], tag=f"w{t}")
                nc.gpsimd.dma_start(
                    wt[:], W[t].rearrange("(p j) m -> p (j m)", p=P))
                wts.append(wt)

            import contextlib

            state = {"dma_i": 0}

            def body():
                # Sequential per tensor, V first. Interleaving V's fp32
                # matmul chains with f32r chains (round-robin order) breaks
                # the fp32 two-pass pairing on the PE and silently degrades
                # V's precision to ~4e-5 rms -- keep each tensor's chains
                # contiguous. V's slower fp32 PE bursts overlap the Q/K
                # loads that run ahead on the other DMA ring/buffers.
                for t, Xd in enumerate((Xv, Xq, Xk)):
                    for g in range(ng):
                        # Per pair a [128, J*DK] block: partition p holds seq
                        # rows p*J..p*J+J-1, so every DMA moves 8KB-contiguous
                        # runs per partition (max descriptor efficiency).
                        xg = xp.tile([P, group * J * DK], tensor_dts[t],
                                     tag=f"x{tensor_dts[t]}")
                        if fused:
                            eng = dma_engines[state["dma_i"] % len(dma_engines)]
                            state["dma_i"] += 1
                            eng.dma_start(
                                xg[:].rearrange("p (q f) -> p q f", q=group),
                                Xd[g * group:(g + 1) * group].rearrange(
                                    "q (p j) d -> p q (j d)", p=P),
                            )
                        else:
                            for q in range(group):
                                eng = dma_engines[state["dma_i"] % len(dma_engines)]
                                state["dma_i"] += 1
                                eng.dma_start(
                                    xg[:, q * J * DK:(q + 1) * J * DK],
                                    Xd[g * group + q].rearrange(
                                        "(p j) d -> p (j d)", p=P),
                                )
                        xg4 = xg.rearrange("p (q j d) -> p q j d", q=group, j=J)
                        ps = pp.tile([M, group * DK], F32)
                        for j in range(J):
                            # out[m,(q,d)] += sum_p W[t,p*J+j,m] * X[q,p*J+j,d]
                            nc.tensor.matmul(
                                ps[:],
                                wts[t][:, j * M:(j + 1) * M],
                                xg4[:, :, j, :],
                                start=(j == 0),
                                stop=(j == J - 1),
                            )
                        so = op.tile([M, group * DK], F32)
                        nc.vector.tensor_copy(so[:], ps[:])
                        nc.sync.dma_start(O[t, g], so[:])

            loop_cm = (tc.For_i(0, loop_reps, 1) if loop_reps > 1
                       else contextlib.nullcontext())
            with loop_cm:
                for it in range(n_iters):
                    body()

    nc.compile()
    _NC_CACHE[key] = nc
    return nc


# test.py introspection: exec_time_ns of the last run when tracing is on.
LAST_RESULTS = None


def _softmax(x, axis=-1):
    x = x - x.max(axis=axis, keepdims=True)
    e = np.exp(x)
    return e / e.sum(axis=axis, keepdims=True)


def _log_softmax(x, axis=-1):
    x = x - x.max(axis=axis, keepdims=True)
    return x - np.log(np.exp(x).sum(axis=axis, keepdims=True))


def kernel(**inputs):
    global LAST_RESULTS
    nc = _build_nc()

    f32 = np.float32
    Q = np.ascontiguousarray(inputs["Q"], dtype=f32).reshape(NPAIR, KTOT, DK)
    K = np.ascontiguousarray(inputs["K"], dtype=f32).reshape(NPAIR, KTOT, DK)
    V = np.ascontiguousarray(inputs["V"], dtype=f32).reshape(NPAIR, KTOT, DK)

    # device-side tensor order is (V, Q, K) -- see _build_nc
    Wt = np.empty((3, KTOT, M), dtype=f32)
    Wt[0] = np.ascontiguousarray(inputs["shrink_v_w"], dtype=f32).T
    Wt[1] = np.ascontiguousarray(inputs["shrink_q_w"], dtype=f32).T
    Wt[2] = np.ascontiguousarray(inputs["shrink_k_w"], dtype=f32).T

    in_maps = [
        {
            "Xv": V[c * PPC:(c + 1) * PPC],
            "Xq": Q[c * PPC:(c + 1) * PPC],
            "Xk": K[c * PPC:(c + 1) * PPC],
            "W": Wt,
        }
        for c in range(N_CORES)
    ]

    try:
        res = bass_utils.run_bass_kernel_spmd(nc, in_maps, list(range(N_CORES)))
    except Exception:
        # one retry for transient device/transport errors
        res = bass_utils.run_bass_kernel_spmd(nc, in_maps, list(range(N_CORES)))
    LAST_RESULTS = res

    # [cores, 3, ng, M, group, DK] -> [3, B, H, M, DK]; t order is (V, Q, K)
    Os = np.stack([res.results[c]["O"] for c in range(N_CORES)])
    per_t = Os.transpose(1, 0, 2, 4, 3, 5).reshape(3, NPAIR, M, DK)
    per_t = per_t.reshape(3, B, H, M, DK).astype(np.float64)

    Vs = per_t[0] + np.asarray(inputs["shrink_v_b"], np.float64)[None, None, :, None]
    Qs = per_t[1] + np.asarray(inputs["shrink_q_b"], np.float64)[None, None, :, None]
    Ks = per_t[2] + np.asarray(inputs["shrink_k_b"], np.float64)[None, None, :, None]

    pk_w = np.asarray(inputs["pk_w"], np.float64)
    pk_b = np.asarray(inputs["pk_b"], np.float64)
    pq_w = np.asarray(inputs["pq_w"], np.float64)
    pq_b = np.asarray(inputs["pq_b"], np.float64)
    qp_w = np.asarray(inputs["qp_w"], np.float64)
    qp_b = np.asarray(inputs["qp_b"], np.float64)
    kp_w = np.asarray(inputs["kp_w"], np.float64)
    kp_b = np.asarray(inputs["kp_b"], np.float64)

    ckp = np.maximum(Ks.reshape(B, -1) @ pk_w.T + pk_b, 0.0)
    cqp = np.maximum(Qs.reshape(B, -1) @ pq_w.T + pq_b, 0.0)
    cluster_q = _softmax(cqp @ qp_w.T + qp_b, axis=-1)
    cluster_k = _softmax(ckp @ kp_w.T + kp_b, axis=-1)

    mu = cluster_q.mean(axis=0)
    sigma = np.logaddexp(0.0, cluster_k.std(axis=0, ddof=1))  # softplus
    logp = (-0.5 * np.square((cluster_k - mu) / sigma)
            - np.log(sigma) - 0.5 * np.log(2.0 * np.pi))
    ce = np.mean(-np.sum(cluster_q * _log_softmax(cluster_q, axis=-1), axis=-1))
    loss = np.asarray(-np.mean(logp) + ce, dtype=f32)

    # Every attention row is uniform 1/8 (see module docstring), so the
    # context is mean_m(Vs) broadcast over all L query positions.
    ctx_row = Vs.mean(axis=2).astype(f32)  # [B, H, DK]
    context = np.ascontiguousarray(
        np.broadcast_to(ctx_row[:, :, None, :], (B, H, L, DK)))

    return context, loss
